# revision 18
# baseline (speedup 1.0000x reference)
"""Trainium2 Bass kernel: matvec, two-engine drain (v13).

scores = encoder_out[16384, 4096] @ decoder_hidden[-1][4096] -> [16384]
Sharding: encoder_out row-wise across 8 cores (2048 rows each);
decoder_hidden's top row host-cast to bf16 [1, 4096], replicated.

Measured facts this build encodes:
  - Single SWDGE queue with fp32->bf16 cast sustains ~420-424 GB/s =
    the per-core DMA fabric cap (~435 GB/s). Adding HWDGE queues LOWERS
    aggregate throughput (arbitration), so all 32 MB rides one queue:
    stream ~79 us. Exec = head + stream + drain-tail.
  - DVE op rates on [128, 4096]: fused scalar_tensor_tensor
    (mul + rowsum accum in one pass) = 4.42 us regardless of operand
    location; plain tensor_mul = 2.29 us with bf16 SBUF operands but
    4.42 us if an operand sits in PSUM (1 elem/cycle PSUM read).
    ACT copy-accum reduce = 4.0 us. No single engine holds the
    4.72 us/block DMA pace, so blocks alternate two drain paths:
      even i: DVE fused STT, in1 = t in PSUM (no tb dependency)
      odd  i: DVE tensor_mul (in1 = tb bf16 SBUF) -> ACT reduce
    Per block-pair: DVE ~7.2 us, ACT ~4.3 us << 9.44 us of DMA.
  - t broadcast: PE ones-matmul into PSUM (8 KB fabric); ACT copies
    PSUM -> SBUF bf16 tb for the mul path (off every critical path).
  - STT/mul outputs written in place: a [128,1] broadcast-out
    hot-spots one SBUF bank and throttles concurrent DMA writes
    (stream 333 vs 424 GB/s measured).
  - Tail DMAs get a FRESH semaphore per chunk: concurrent chunk
    transfers increment a shared semaphore out of order, so
    cumulative waits can pass before a given chunk's data is in SBUF
    (this was a real, observed race: block-15 errors up to 2e-1).
  - PSUM column slices kept 2 KB-bank aligned.
  - Tail: b14 via ACT path in halves, b15 fused in chunks
    (1536, 1024, 1024, 512) so both engines drain concurrently and
    the last chunk's compute is short.
  - Final store from sync HWDGE (a trailing SWDGE store made the
    gpsimd end-of-stream DRAIN wait ~2.4 us); cols 0:12 stored early.

Output sc [128, 20]: cols 0..13 = b0..b13; 14,15 = b14 halves;
16..19 = b15 chunks. Host: blk14 = c14+c15, blk15 = sum(c16..c19).

Accuracy: bf16 products, fp32 accumulation -> rel err ~3e-3 << 2e-2.
"""

import numpy as np

S, H, L = 16384, 4096, 2
N_CORES = 8
S_LOC = S // N_CORES        # 2048
P = 128
N_BLOCKS = S_LOC // P       # 16
NBUF = 12
HH = H // 2                 # 2048
NCOL = 20

_NC_CACHE = {}
LAST_RESULT = None


def _build_nc():
    import concourse.bass as bass
    from concourse import mybir

    f32 = mybir.dt.float32
    bf16 = mybir.dt.bfloat16
    MUL = mybir.AluOpType.mult

    nc = bass.Bass(trn_type="TRN2")
    enc = nc.dram_tensor("enc", [S_LOC, H], f32, kind="ExternalInput")
    tbd = nc.dram_tensor("tbd", [1, H], bf16, kind="ExternalInput")
    out = nc.dram_tensor("out", [P, NCOL], f32, kind="ExternalOutput")

    enc_r = enc.rearrange("(n p) h -> n p h", p=P)

    NFULL = N_BLOCKS - 2            # 14 full blocks
    is_fused = [i % 2 == 0 for i in range(NFULL)]   # even -> DVE fused

    def fused_upto(i):
        return sum(1 for k in range(i + 1) if is_fused[k])

    def act_upto(i):
        return sum(1 for k in range(i + 1) if not is_fused[k])

    N_FUSED = sum(is_fused)          # 7
    N_ACT = NFULL - N_FUSED          # 7

    # tail schedule (offline-searched, block-grouped): 'F' = DVE fused
    # STT, 'A' = DVE mul -> ACT reduce. (block, lo, hi, path, sc_col)
    TAIL = [
        (14, 0, HH, "A", 14),
        (14, HH, H, "F", 15),
        (15, 0, 1536, "A", 16),
        (15, 1536, 2560, "A", 17),
        (15, 2560, 3072, "F", 18),
        (15, 3072, H, "F", 19),
    ]

    from contextlib import ExitStack

    with ExitStack() as ctx:
        t16 = ctx.enter_context(nc.sbuf_tensor("t16", [1, H], bf16))
        ones = ctx.enter_context(nc.sbuf_tensor("ones", [1, P], bf16))
        tbps = ctx.enter_context(nc.psum_tensor("tbps", [P, H], f32))
        tb = ctx.enter_context(nc.sbuf_tensor("tb", [P, H], bf16))
        ebufs = [
            ctx.enter_context(nc.sbuf_tensor(f"ebuf{i}", [P, H], bf16))
            for i in range(NBUF)
        ]
        junk = ctx.enter_context(nc.sbuf_tensor("junk", [P, H], bf16))
        sc = ctx.enter_context(nc.sbuf_tensor("sc", [P, NCOL], f32))
        t_sem = ctx.enter_context(nc.semaphore("t_sem"))
        pe_sem = ctx.enter_context(nc.semaphore("pe_sem"))
        tb_sem = ctx.enter_context(nc.semaphore("tb_sem"))
        esems = [ctx.enter_context(nc.semaphore(f"esem{i}")) for i in range(NBUF)]
        tsems = [ctx.enter_context(nc.semaphore(f"tsem{i}")) for i in range(6)]
        mul_sem = ctx.enter_context(nc.semaphore("mul_sem"))
        redv_sem = ctx.enter_context(nc.semaphore("redv_sem"))
        reda_sem = ctx.enter_context(nc.semaphore("reda_sem"))
        store_sem = ctx.enter_context(nc.semaphore("store_sem"))
        block = ctx.enter_context(nc.Block())

        @block.gpsimd
        def _(gpsimd):
            # b0..b13 full tiles; b0's dma_start is the FIRST instruction
            for i in range(NFULL):
                if i >= NBUF:
                    j = i - NBUF      # block whose slot we reuse
                    if is_fused[j]:
                        gpsimd.wait_ge(redv_sem, fused_upto(j))
                    else:
                        gpsimd.wait_ge(reda_sem, act_upto(j))
                gpsimd.dma_start(ebufs[i % NBUF][:], enc_r[i]).then_inc(
                    esems[i % NBUF], 16
                )
            # tail pieces: b14 -> slot 6, b15 -> slot 7 (both single-use
            # slots with NBUF=12); fresh sem per piece
            gpsimd.wait_ge(redv_sem, fused_upto(6))
            gpsimd.wait_ge(reda_sem, act_upto(7))
            for k, (blk, lo, hi, path, col) in enumerate(TAIL):
                slot = 6 if blk == 14 else 7
                gpsimd.dma_start(
                    ebufs[slot][:, lo:hi], enc_r[blk, :, lo:hi]
                ).then_inc(tsems[k], 16)

        @block.tensor
        def _(tensor):
            tensor.wait_ge(t_sem, 17)
            for j in range(8):
                mm = nc.tensor.matmul(
                    tbps[:, j * 512 : (j + 1) * 512],
                    ones[:],
                    t16[:, j * 512 : (j + 1) * 512],
                    start=True,
                    stop=True,
                )
            mm.then_inc(pe_sem, 1)

        @block.vector
        def _(vector):
            nc.vector.memset(ones[:], 1.0).then_inc(t_sem, 1)
            vector.wait_ge(pe_sem, 1)
            for i in range(NFULL):
                vector.wait_ge(esems[i % NBUF], 16 * (i // NBUF + 1))
                eb = ebufs[i % NBUF][:]
                if is_fused[i]:
                    nc.vector.scalar_tensor_tensor(
                        out=eb, in0=eb, scalar=1.0, in1=tbps[:],
                        op0=MUL, op1=MUL,
                        accum_out=sc[:, i : i + 1],
                    ).then_inc(redv_sem, 1)
                else:
                    if i == 1:
                        vector.wait_ge(tb_sem, 1)
                    nc.vector.tensor_mul(eb, eb, tb[:]).then_inc(mul_sem, 1)
            # tail pieces in landing order
            for k, (blk, lo, hi, path, col) in enumerate(TAIL):
                slot = 6 if blk == 14 else 7
                vector.wait_ge(tsems[k], 16)
                eb = ebufs[slot][:, lo:hi]
                if path == "F":
                    nc.vector.scalar_tensor_tensor(
                        out=eb, in0=eb, scalar=1.0, in1=tbps[:, lo:hi],
                        op0=MUL, op1=MUL,
                        accum_out=sc[:, col : col + 1],
                    ).then_inc(redv_sem, 1)
                else:
                    nc.vector.tensor_mul(
                        eb, eb, tb[:, lo:hi]
                    ).then_inc(mul_sem, 1)

        @block.scalar
        def _(scalar):
            # warm the ACT function table while idle
            nc.scalar.activation(
                out=junk[0:1, 0:1],
                in_=junk[0:1, 0:1],
                func=mybir.ActivationFunctionType.Copy,
            )
            # materialize tb (bf16 SBUF) for the mul path
            scalar.wait_ge(pe_sem, 1)
            nc.scalar.activation(
                out=tb[:],
                in_=tbps[:],
                func=mybir.ActivationFunctionType.Copy,
            ).then_inc(tb_sem, 1)
            nmul = 0
            for i in range(NFULL):
                if is_fused[i]:
                    continue
                nmul += 1
                scalar.wait_ge(mul_sem, nmul)
                nc.scalar.activation(
                    out=junk[:],
                    in_=ebufs[i % NBUF][:],
                    func=mybir.ActivationFunctionType.Copy,
                    accum_out=sc[:, i : i + 1],
                ).then_inc(reda_sem, 1)
            # tail A-pieces in mul order
            for blk, lo, hi, path, col in TAIL:
                if path != "A":
                    continue
                slot = 6 if blk == 14 else 7
                nmul += 1
                scalar.wait_ge(mul_sem, nmul)
                nc.scalar.activation(
                    out=junk[:, lo:hi],
                    in_=ebufs[slot][:, lo:hi],
                    func=mybir.ActivationFunctionType.Copy,
                    accum_out=sc[:, col : col + 1],
                ).then_inc(reda_sem, 1)

        @block.sync
        def _(sync):
            sync.dma_start(t16[:], tbd[0:1, :]).then_inc(t_sem, 16)
            # early store of cols 0:12 (desc-gen overlaps the stream)
            sync.wait_ge(redv_sem, fused_upto(11))
            sync.wait_ge(reda_sem, act_upto(11))
            sync.dma_start(out[:, 0:12], sc[:, 0:12]).then_inc(store_sem, 16)
            sync.wait_ge(redv_sem, N_FUSED + 3)
            sync.wait_ge(reda_sem, N_ACT + 3)
            sync.dma_start(out[:, 12:NCOL], sc[:, 12:NCOL]).then_inc(store_sem, 16)

    return nc


def kernel(encoder_out: np.ndarray, decoder_hidden: np.ndarray) -> np.ndarray:
    global LAST_RESULT
    import ml_dtypes
    from concourse.bass_utils import run_bass_kernel_spmd

    encoder_out = np.ascontiguousarray(np.asarray(encoder_out, dtype=np.float32))
    decoder_hidden = np.ascontiguousarray(np.asarray(decoder_hidden, dtype=np.float32))

    if "nc" not in _NC_CACHE:
        _NC_CACHE["nc"] = _build_nc()
    nc = _NC_CACHE["nc"]

    tb_np = np.ascontiguousarray(
        decoder_hidden[L - 1 : L].astype(ml_dtypes.bfloat16))

    in_maps = [
        {"enc": encoder_out[c * S_LOC : (c + 1) * S_LOC], "tbd": tb_np}
        for c in range(N_CORES)
    ]
    res = run_bass_kernel_spmd(nc, in_maps, core_ids=list(range(N_CORES)))
    LAST_RESULT = res

    parts = []
    for r in res.results:
        sc = np.asarray(r["out"])  # [128, 20]
        blk = np.concatenate(
            [
                sc[:, 0:14],
                (sc[:, 14] + sc[:, 15])[:, None],       # block 14
                sc[:, 16:20].sum(axis=1)[:, None],       # block 15
            ],
            axis=1,
        )  # [128, 16]
        parts.append(blk.T.reshape(-1))
    return np.concatenate(parts).astype(np.float32)
